# revision 2
# baseline (speedup 1.0000x reference)
"""BinaryTreeGRU Trainium2 kernel (all-bf16 pipeline).

Batch of B=64 complete binary trees (L=512 leaves, 1023 nodes each),
data-parallel over trees across 8 NeuronCores (8 trees/core).

Layout: feature-major "folded": activations live in SBUF as
[128 partitions, 2 feature-blocks, n_cols] bf16 (mem 256 = 2 blocks).
Level h storage is parity-split ([128, cb, par, n/2]) so that the next
level reads children (left/right) as contiguous column ranges.

Everything is bf16 (weights, moving operands, gates, h, DRAM in/out);
PSUM accumulation stays f32.  bf16 weights enable the PE's automatic
fast-weight-load path (fp32r blocks it) so LDWEIGHTS hides behind the
matmul stream.

Per internal chunk (NC=256 cols):
  PE : 32 rzh matmuls (4 quarters x 2 blocks x 4 kc) + 4 Wgh matmuls
  ACT: sigmoid on the (r_l,r_r) PSUM pair, sigmoid on (z_l,z_r), tanh
  DVE: rp=r*ch, s=rp_l+rp_r, zz=z*ch, zh=zz_l+zz_r, zs=z_l+z_r,
       tt=1-zs/2 (tensor_scalar, 4x mode), v=tt*g
  GpS: h = v + zh written parity-split (strided write is free there)
"""

import os
from contextlib import ExitStack

import numpy as np
import ml_dtypes

import concourse.bass as bass
import concourse.mybir as mybir
import concourse.tile as tile
from concourse import bacc
from concourse.bass_utils import run_bass_kernel_spmd

F32 = mybir.dt.float32
BF16 = mybir.dt.bfloat16
MULT = mybir.AluOpType.mult
ADD = mybir.AluOpType.add
SIGMOID = mybir.ActivationFunctionType.Sigmoid
TANH = mybir.ActivationFunctionType.Tanh

MEM = 256
IN_DIM = 256
B = 64
L = 512
NCORES = 8
BLOC = B // NCORES            # trees per core
N0 = BLOC * L                 # leaf columns per core = 4096
NLEVELS = 10                  # 4096,2048,...,8 columns
NCOLS = [N0 >> l for l in range(NLEVELS)]
TOT = sum(NCOLS)              # 8184
OFFS = np.cumsum([0] + NCOLS).tolist()
NC = 256                      # node-column chunk

LAST_RESULT = {}


def _wavefront_order(nchunks, d=2):
    """Topological chunk order interleaving levels (see baseline)."""
    pos = {}
    order = []
    remaining = [(lv, ci) for lv in range(NLEVELS)
                 for ci in range(nchunks[lv])]

    def parents(lv, ci):
        if lv == 0:
            return []
        if nchunks[lv - 1] == 2 * nchunks[lv]:
            return [(lv - 1, 2 * ci), (lv - 1, 2 * ci + 1)]
        return [(lv - 1, pc) for pc in range(nchunks[lv - 1])]

    t = 0
    while remaining:
        ready = []
        for (lv, ci) in remaining:
            ps = parents(lv, ci)
            dd = d if lv > 1 else (d if lv == 1 else 0)
            if all(p in pos and pos[p] + (2 if lv == 1 else dd + 1) <= t
                   for p in ps):
                ready.append((lv, ci))
        if ready:
            ch = max(ready, key=lambda c: (c[0], -c[1]))
            pos[ch] = t
            order.append(ch)
            remaining.remove(ch)
        else:
            order.append(None)   # spacing step (emit only a back)
        t += 1
    return order


def build_nc(fast_bias: bool):
    nc = bacc.Bacc("TRN2", target_bir_lowering=False, debug=False)

    d_x = nc.dram_tensor("xT", [128, 2, N0], BF16, kind="ExternalInput")
    d_wrzh = nc.dram_tensor("wrzh", [4, 128, 1024], BF16, kind="ExternalInput")
    d_wgrzx = nc.dram_tensor("wgrzx", [2, 128, 768], BF16, kind="ExternalInput")
    d_wgh = nc.dram_tensor("wgh", [2, 128, 256], BF16, kind="ExternalInput")
    d_bias = nc.dram_tensor("bias6", [6, 128, 1], F32, kind="ExternalInput")
    d_out = nc.dram_tensor("out", [2, 128, TOT], BF16, kind="ExternalOutput")

    x = d_x.ap()
    wrzh = d_wrzh.ap()
    wgrzx = d_wgrzx.ap()
    wgh = d_wgh.ap()
    bias6 = d_bias.ap()
    out = d_out.ap()

    mm = nc.tensor.matmul
    nchunks = [max(1, NCOLS[lv] // NC) for lv in range(NLEVELS)]

    with tile.TileContext(nc) as tc, ExitStack() as ctx:
        singles = ctx.enter_context(tc.tile_pool(name="singles", bufs=1))
        xpool = ctx.enter_context(tc.tile_pool(name="xpool", bufs=3))
        gates_pool = ctx.enter_context(tc.tile_pool(name="gates", bufs=2))
        spool = ctx.enter_context(tc.tile_pool(name="spool", bufs=3))
        scratch = ctx.enter_context(tc.tile_pool(name="scratch", bufs=2))
        psum = ctx.enter_context(tc.tile_pool(name="psum", bufs=3, space="PSUM"))
        psumg = ctx.enter_context(tc.tile_pool(name="psumg", bufs=2, space="PSUM"))

        # --- load constants (wgrzx first: the leaf phase needs only it) ---
        w_grzx = []
        for kc in range(2):
            t = singles.tile([128, 768], BF16, tag=f"wgrzx{kc}", name=f"wgrzx{kc}")
            nc.sync.dma_start(out=t, in_=wgrzx[kc])
            w_grzx.append(t)
        b_t = []
        if not fast_bias:
            for i in range(6):
                t = singles.tile([128, 1], F32, tag=f"b{i}", name=f"b{i}")
                nc.sync.dma_start(out=t, in_=bias6[i])
                b_t.append(t)

        # x chunks: loaded per leaf chunk from [128, 2, N0]
        x_tiles = {}

        def load_x(ci):
            t = xpool.tile([128, 2, NC], BF16, tag="x", name=f"x{ci}")
            nc.sync.dma_start(out=t, in_=x[:, :, ci * NC:(ci + 1) * NC])
            x_tiles[ci] = t

        # prefetch first few x chunks before the deferred weight loads
        for ci in range(4):
            load_x(ci)

        w_rzh = []
        for kc in range(4):
            t = singles.tile([128, 1024], BF16, tag=f"wrzh{kc}", name=f"wrzh{kc}")
            nc.sync.dma_start(out=t, in_=wrzh[kc])
            w_rzh.append(t)
        w_gh = []
        for kc in range(2):
            t = singles.tile([128, 256], BF16, tag=f"wgh{kc}", name=f"wgh{kc}")
            nc.sync.dma_start(out=t, in_=wgh[kc])
            w_gh.append(t)

        h_t = [singles.tile([128, 2, 2, max(1, NCOLS[l] // 2)], BF16,
                            tag=f"h{l}", name=f"h{l}", bufs=1)
               for l in range(NLEVELS)]

        def h_scatter(lv, c0, ncur):
            """Parity-scatter write view of h_t[lv] covering natural columns
            c0..c0+ncur: dims (cb, j, par), par innermost (stride=half)."""
            t = h_t[lv]
            half = max(1, NCOLS[lv] // 2)
            j0 = c0 // 2
            n2 = ncur // 2
            return bass.AP(tensor=t.tensor, offset=t.offset + j0,
                           ap=[list(t.ap[0]), [2 * half, 2], [1, n2],
                               [half, 2]])

        def sig(dst, ps_tile, par_blocks):
            """sigmoid(ps + 1) -> dst.  ps_tile [128, 2, 2, n] (par, cb, n),
            dst view [128, 2(cb), 2(par), n] slices.  par_blocks lists the
            (par,) coords for the fallback per-block bias path."""
            if fast_bias:
                nc.scalar.activation(dst, ps_tile, SIGMOID, bias=1.0)
            else:
                for i, par in enumerate(par_blocks):
                    for cb in range(2):
                        nc.scalar.activation(
                            dst[:, cb, i, :], ps_tile[:, i, cb, :],
                            SIGMOID, bias=b_t[2 + 2 * par + cb])

        state = {}   # (lv, ci) -> dict for the back phase

        def emit_leaf_front(ci):
            n = NC
            c0 = ci * n
            if ci + 4 < nchunks[0]:
                load_x(ci + 4)
            x_c = x_tiles.pop(ci)
            # z-gates PSUM pair: [128, par(2), cb(2), n]
            ps = psum.tile([128, 2, 2, n], F32, tag="P", name="ps_z")
            for par in range(2):
                for cb in range(2):
                    col = 256 + par * 256 + cb * 128
                    for kc in range(2):
                        mm(ps[:, par, cb, :], w_grzx[kc][:, col:col + 128],
                           x_c[:, kc, :], start=(kc == 0), stop=(kc == 1))
            psg = psumg.tile([128, 2, n], F32, tag="G", name="ps_gx")
            for cb in range(2):
                for kc in range(2):
                    mm(psg[:, cb, :], w_grzx[kc][:, cb * 128:cb * 128 + 128],
                       x_c[:, kc, :], start=(kc == 0), stop=(kc == 1))
            # gates
            gz = gates_pool.tile([128, 2, 2, n], BF16, tag="gz", name="gz")
            # gz dims (cb, par, n); sigmoid writes both par planes
            if fast_bias:
                nc.scalar.activation(
                    bass.AP(tensor=gz.tensor, offset=gz.offset,
                            ap=[list(gz.ap[0]), [n, 2], [2 * n, 2], [1, n]]),
                    ps, SIGMOID, bias=1.0)
            else:
                sig(gz, ps, (0, 1))
            tg = scratch.tile([128, 2, n], BF16, tag="tg", name="tg")
            if fast_bias:
                nc.scalar.activation(tg, psg, TANH, bias=0.0)
            else:
                for cb in range(2):
                    nc.scalar.activation(tg[:, cb, :], psg[:, cb, :],
                                         TANH, bias=b_t[cb])
            zs = scratch.tile([128, 2, n], BF16, tag="zs", name="zs")
            nc.vector.tensor_add(zs, gz[:, :, 0, :], gz[:, :, 1, :])
            tt = scratch.tile([128, 2, n], BF16, tag="tt", name="tt")
            nc.vector.tensor_scalar(tt, zs, -0.5, 1.0, MULT, ADD)
            # h = tt * tg  (parity-scatter write on gpsimd)
            nc.gpsimd.tensor_mul(h_scatter(0, c0, n), tt, tg)
            if ci == nchunks[0] - 1:
                for cb in range(2):
                    nc.sync.dma_start(out=out[cb, :, OFFS[0]:OFFS[1]],
                                      in_=h_t[0][:, cb, :, :])

        def emit_front(lv, ci):
            if lv == 0:
                emit_leaf_front(ci)
                return
            n = min(NCOLS[lv], NC)
            c0 = ci * n
            hp = h_t[lv - 1]
            hps = hp[:, :, :, c0:c0 + n]          # [128, cb, par, n]

            gr = gates_pool.tile([128, 2, 2, n], BF16, tag="gr", name="gr")
            gz = gates_pool.tile([128, 2, 2, n], BF16, tag="gz", name="gz")
            for rz, dst in ((0, gr), (1, gz)):
                ps = psum.tile([128, 2, 2, n], F32, tag="P", name=f"ps{rz}")
                for par in range(2):
                    for cb in range(2):
                        col = rz * 512 + par * 256 + cb * 128
                        for kc in range(4):
                            mm(ps[:, par, cb, :],
                               w_rzh[kc][:, col:col + 128],
                               hp[:, kc % 2, kc // 2, c0:c0 + n],
                               start=(kc == 0), stop=(kc == 3))
                if fast_bias:
                    # write [cb, par] planes from ps's (par, cb) layout
                    nc.scalar.activation(
                        bass.AP(tensor=dst.tensor, offset=dst.offset,
                                ap=[list(dst.ap[0]), [n, 2], [2 * n, 2],
                                    [1, n]]),
                        ps, SIGMOID, bias=1.0)
                else:
                    sig(dst, ps, (0, 1))

            # r-path: s = r_l*h_l + r_r*h_r
            rp = scratch.tile([128, 2, 2, n], BF16, tag="rp", name="rp")
            nc.vector.tensor_mul(rp, gr, hps)
            s = spool.tile([128, 2, n], BF16, tag="s", name="s")
            nc.vector.tensor_add(s, rp[:, :, 0, :], rp[:, :, 1, :])
            # z-path
            zz = scratch.tile([128, 2, 2, n], BF16, tag="zz", name="zz")
            nc.vector.tensor_mul(zz, gz, hps)
            zh = spool.tile([128, 2, n], BF16, tag="zh", name="zh")
            nc.vector.tensor_add(zh, zz[:, :, 0, :], zz[:, :, 1, :])
            zs = scratch.tile([128, 2, n], BF16, tag="zs", name="zs")
            nc.vector.tensor_add(zs, gz[:, :, 0, :], gz[:, :, 1, :])
            tt = spool.tile([128, 2, n], BF16, tag="tt", name="tt")
            nc.vector.tensor_scalar(tt, zs, -0.5, 1.0, MULT, ADD)
            state[(lv, ci)] = dict(s=s, zh=zh, tt=tt, c0=c0, ncur=n)

        def emit_back(lv, ci):
            if lv == 0:
                return
            st = state.pop((lv, ci))
            s, zh, tt = st["s"], st["zh"], st["tt"]
            c0, n = st["c0"], st["ncur"]

            psg = psumg.tile([128, 2, n], F32, tag="G", name="ps_g")
            for mb in range(2):
                for kc in range(2):
                    mm(psg[:, mb, :], w_gh[kc][:, 128 * mb:128 * mb + 128],
                       s[:, kc, :], start=(kc == 0), stop=(kc == 1))
            g_sb = scratch.tile([128, 2, n], BF16, tag="gsb", name="g_sb")
            if fast_bias:
                nc.scalar.activation(g_sb, psg, TANH, bias=0.0)
            else:
                for cb in range(2):
                    nc.scalar.activation(g_sb[:, cb, :], psg[:, cb, :],
                                         TANH, bias=b_t[cb])
            v = scratch.tile([128, 2, n], BF16, tag="v", name="v")
            nc.vector.tensor_mul(v, tt, g_sb)
            # h = v + zh (parity-scatter write on gpsimd)
            nc.gpsimd.tensor_add(h_scatter(lv, c0, n), v, zh)
            if ci == nchunks[lv] - 1:
                for cb in range(2):
                    nc.sync.dma_start(out=out[cb, :, OFFS[lv]:OFFS[lv + 1]],
                                      in_=h_t[lv][:, cb, :])

        D = 2
        order = _wavefront_order(nchunks, D)

        def parent_list(lv, ci):
            if lv == 0:
                return []
            if nchunks[lv - 1] == 2 * nchunks[lv]:
                return [(lv - 1, 2 * ci), (lv - 1, 2 * ci + 1)]
            return [(lv - 1, pc) for pc in range(nchunks[lv - 1])]

        pending = []
        done = set()

        def pop_back():
            b = pending.pop(0)
            emit_back(*b)
            done.add(b)

        for ch in order:
            if ch is None:
                if pending:
                    pop_back()
                continue
            lv, ci = ch
            for par in parent_list(lv, ci):
                while par not in done:
                    pop_back()
            emit_front(lv, ci)
            pending.append(ch)
            while len(pending) > D:
                pop_back()
        while pending:
            pop_back()

    nc.compile()
    return nc


def _prep_inputs(inputs, Wgrzx, bgrzx, Wrzh, Wgh):
    """Host-side shard + layout prep. Returns (in_maps, fast_bias)."""
    x = np.ascontiguousarray(inputs, dtype=np.float32)
    Wgrzx = np.asarray(Wgrzx, dtype=np.float32)
    bgrzx = np.asarray(bgrzx, dtype=np.float32)
    Wrzh = np.asarray(Wrzh, dtype=np.float32)
    Wgh = np.asarray(Wgh, dtype=np.float32)

    fast_bias = bool(
        np.all(bgrzx[:MEM] == 0.0) and np.all(bgrzx[MEM:] == 1.0))

    bf = ml_dtypes.bfloat16
    # [256, 768] -> [2, 128, 768]
    wgrzxT = np.ascontiguousarray(Wgrzx.T.reshape(2, 128, 768)).astype(bf)
    # [512, 1024] natural block order (l0, l1, r0, r1)
    wrzhT = np.ascontiguousarray(Wrzh.T.reshape(4, 128, 1024)).astype(bf)
    wghT = np.ascontiguousarray(Wgh.T.reshape(2, 128, 256)).astype(bf)
    bias6 = np.ascontiguousarray(bgrzx.reshape(6, 128, 1))

    in_maps = []
    for c in range(NCORES):
        xc = x[c * BLOC:(c + 1) * BLOC].reshape(N0, IN_DIM)
        # [N0, 256] -> T -> [2, 128, N0] -> [128, 2, N0]
        xT = np.ascontiguousarray(
            xc.T.reshape(2, 128, N0).transpose(1, 0, 2)).astype(bf)
        in_maps.append({
            "xT": xT,
            "wrzh": wrzhT,
            "wgrzx": wgrzxT,
            "wgh": wghT,
            "bias6": bias6,
        })
    return in_maps, fast_bias


def _gather(results):
    """results: list of per-core {'out': [2,128,TOT]} -> [B, 2L-1, MEM]."""
    outs = []
    for c in range(len(results)):
        fm = np.asarray(results[c]["out"]).astype(np.float32).reshape(MEM, TOT)
        levels = []
        for lv in range(NLEVELS):
            n = NCOLS[lv]
            blk = fm[:, OFFS[lv]:OFFS[lv + 1]]
            nat = np.empty_like(blk)
            nat[:, 0::2] = blk[:, :n // 2]
            nat[:, 1::2] = blk[:, n // 2:]
            k = n // BLOC
            levels.append(nat.reshape(MEM, BLOC, k).transpose(1, 2, 0))
        outs.append(np.concatenate(levels, axis=1))
    return np.ascontiguousarray(
        np.concatenate(outs, axis=0), dtype=np.float32)


def kernel(**inputs):
    in_maps, fast_bias = _prep_inputs(
        inputs["inputs"], inputs["Wgrzx"], inputs["bgrzx"],
        inputs["Wrzh"], inputs["Wgh"])
    nc = build_nc(fast_bias)
    trace = bool(int(os.environ.get("BTGRU_TRACE", "0")))
    res = run_bass_kernel_spmd(
        nc, in_maps, core_ids=list(range(NCORES)), trace=trace)
    LAST_RESULT.clear()
    LAST_RESULT["exec_time_ns"] = res.exec_time_ns
    LAST_RESULT["profile_json"] = res.profile_json
    return _gather(res.results)
